# revision 19
# baseline (speedup 1.0000x reference)
"""Trainium2 Bass kernel for the ExpCloudMMD loss (v2: ACT+DVE split).

reference math (gamma = 0.5):
  t1 = mean_{j,k} exp(-g*||p_j - p_k||^2)            over [8192, 8192]
  t2 = 2/(Nx*Np) * sum_{i,j} exp(-g*||x_i - p_j||^2) over [32768, 8192]
  out = t1 - t2  (f32 scalar)

Strategy (8 cores, SPMD, no collectives):
  - t2: shard x rows 8-way (4096 rows/core vs all 8192 particles).
  - t1: coarse triangle over 2048-super-blocks, 160 (row-block, col-group)
    pairs dealt round-robin to cores via the pslhs input (program identical
    across cores); host doubles the off-diagonal sums.
  - The PE emits the *Schraudolph-scaled* exp argument directly:
        z = A*(p.x) - A*g*|p|^2 - A*g*|x|^2 + B,  A = 2^7/ln2,
        B = 127*2^7 - C  (C ~ 7.345 centers the bitcast sawtooth error)
    via one K=68 bf16 matmul per tile (4-way hi/lo split product + norm/
    bias channels).
  - Columns are split between two exp engines running concurrently:
      * ACT waves [128,1024]: activation(Exp, scale=1/A, bias=-B/A,
        accum_out) -- exact exp row-sums (the inverse affine is free).
      * DVE waves [128,1024]: tensor_scalar_max(int16_out, psum, 0)
        clamps and converts z to int16; bitcasting those bits as bf16 IS
        exp(a)*(1+eps), |eps|<~4%, mean-centered by C. A second
        tensor_scalar (bf16, 4x mode) accumulates them via accum_out.
    Wave pattern 9 ACT : 7 DVE balances the engines; PSUM = 2x1024 (ACT
    ping-pong) + 2x1024 (DVE ping-pong).
  - Per-wave row-sums land in one column of a [128, n_cols] SBUF
    accumulator; host does the final tiny weighted reduction in fp64.
    The sawtooth-centering constant C rides in the *input encoding*, so
    it can be recalibrated without recompiling.
"""

import threading

import ml_dtypes
import numpy as np

import concourse.bass as bass  # noqa: F401
import concourse.mybir as mybir
import concourse.tile as tile
from concourse import bacc, bass_utils

bf16 = ml_dtypes.bfloat16

GAMMA = 0.5
NX, NP, D = 32768, 16384 // 2, 16
N_CORES = 8
XS = NX // N_CORES     # 4096 x rows per core
K = 68                 # 4*16 (hi/lo product blocks) + 2 + 2 norm channels
W = 1024               # wave width (columns per exp-engine instruction)

A_SCHR = 128.0 / np.log(2.0)
# sawtooth centering: 7.3453 if the DVE fp32->int16 convert rounds to
# nearest, 6.8458 if it truncates. Rides in the *inputs* (encodings + the
# biasv tensor), so it can be recalibrated without recompiling.
C_SCHR = 7.3453


def _b_schr():
    return 127.0 * 128.0 - C_SCHR

# t1 coarse-triangle schedule: for col-super-group g (2048 particles), the
# computed row-blocks are the 16*(g+1) blocks of super-rows 0..g, dealt
# round-robin (r % 8) to cores -> per-core counts 2,4,6,8. Rows ascend, so
# per level g the first 2g pairs are off-diagonal (w=2) and the last 2 are
# diagonal (w=1) on EVERY core -- the program stays identical across cores.
T1_COUNTS = [2, 4, 6, 8]
N_T1_PAIRS = sum(T1_COUNTS)                    # 20 per core
PS_COLS = N_T1_PAIRS * 128                     # 2560 pslhs columns per core

# engine wave pattern: ratio ACT:DVE per 16-wave window
PAT_ACT, PAT_DVE = 9, 7

N_PCHUNK = 8  # plhs load chunks for early compute start


def _t1_pairs(core):
    """[(row_block, col_group, weight)] for this core, in program order."""
    pairs = []
    for g in range(4):
        rows = [r for r in range(16 * (g + 1)) if r % N_CORES == core]
        assert len(rows) == T1_COUNTS[g]
        for r in rows:
            pairs.append((r, g, 1.0 if r // 16 == g else 2.0))
    return pairs


def _t1_pair_waves():
    """Per t1 pair (in _t1_pairs order): list of (col_start, weight) waves.

    Off-diagonal pairs: full 2048 cols at w=2. The two diagonal pairs per
    level split the symmetric diagonal super-block into quadrants
    (UL + 2*UR + LR): the first diag row-block (i<8) does [0,1024) at w=1
    and [1024,2048) at w=2; the second (i>=8) only [1024,2048) at w=1.
    """
    out = []
    for g in range(4):
        base = g * 2048
        for _ in range(T1_COUNTS[g] - 2):
            out.append([(base, 2.0), (base + W, 2.0)])
        out.append([(base, 1.0), (base + W, 2.0)])
        out.append([(base + W, 1.0)])
    return out


def _wave_stream():
    """Yield (src, slot, col_start, kind, weight, dve_ok) work waves.

    src: 'x' (cross, rhs=xrhs, lhs=plhs[slot]) or 'p' (t1, rhs=prhs,
    lhs=pslhs[slot]). kind: 't2' or 't1'. Cross j-blocks are interleaved
    with t1 pairs (one pair after every 3rd j-block) so neither engine
    sees a long forced tail. Identical across cores by construction.

    t1 waves are ACT-only: the diagonal super-blocks contain the exact
    self-pairs (a point mass at one sawtooth phase of the bitcast-exp
    error, which would not average out on the DVE path).
    """
    t1_seq = _t1_pair_waves()
    t1_idx = 0
    for j in range(NP // 128):
        for c in range(4096 // W):
            yield ("x", j, c * W, "t2", 1.0, True)
        if (j % 3 == 2) and t1_idx < N_T1_PAIRS:
            for cs, w in t1_seq[t1_idx]:
                yield ("p", t1_idx, cs, "t1", w, False)
            t1_idx += 1
    while t1_idx < N_T1_PAIRS:
        for cs, w in t1_seq[t1_idx]:
            yield ("p", t1_idx, cs, "t1", w, False)
        t1_idx += 1


def _schedule():
    """Assign each wave to an engine. Returns (plan, n_cols) where plan is a
    list of (engine, src, slot, col_start, kind, weight) in emission order.
    Forced-ACT waves (w=2 t1) consume ACT turns; the 9:7 pattern otherwise
    alternates to balance engine busy-time."""
    waves = list(_wave_stream())
    plan = []
    # greedy balance of modeled per-wave engine busy-times
    t_act = 1184.0   # (1024+172)/1.2 + 187
    t_dve = 1474.0   # (1024+120)/0.96 + ((1024+58)/0.96)/VBATCH
    act_t = dve_t = 0.0
    for wv in waves:
        dve_ok = wv[5]
        if dve_ok and dve_t + t_dve <= act_t + t_act:
            plan.append(("V",) + wv[:5])
            dve_t += t_dve
        else:
            plan.append(("A",) + wv[:5])
            act_t += t_act
    return plan


VBATCH = 4  # DVE waves per accumulator column
_PLAN = _schedule()


def _colplan():
    """(kind, weight) per accumulator column, in emission order."""
    cols = []
    vcount = 0
    for eng, _src, _slot, _cs, kind, w in _PLAN:
        if eng == "A":
            cols.append((kind, w))
        else:
            vcount += 1
            if vcount == VBATCH:
                cols.append(("t2", 1.0))
                vcount = 0
    if vcount:
        cols.append(("t2", 1.0))
    return cols


_COLS = _colplan()
N_COLS = len(_COLS)


def _build_nc(repeats=1):
    nc = bacc.Bacc(
        "TRN2",
        target_bir_lowering=False,
        debug=False,
        enable_asserts=False,
        num_devices=N_CORES,
    )
    dt = mybir.dt
    plhs = nc.dram_tensor("plhs", [K, NP], dt.bfloat16, kind="ExternalInput").ap()
    prhs = nc.dram_tensor("prhs", [K, NP], dt.bfloat16, kind="ExternalInput").ap()
    xrhs = nc.dram_tensor("xrhs", [K, XS], dt.bfloat16, kind="ExternalInput").ap()
    pslhs = nc.dram_tensor("pslhs", [K, PS_COLS], dt.bfloat16, kind="ExternalInput").ap()
    biasv = nc.dram_tensor("biasv", [128, 1], dt.float32, kind="ExternalInput").ap()
    acc_d = nc.dram_tensor("acc", [128, N_COLS], dt.float32, kind="ExternalOutput").ap()

    inv_a = float(1.0 / A_SCHR)

    with tile.TileContext(nc) as tc:
        with (
            tc.tile_pool(name="const", bufs=1) as const,
            tc.tile_pool(name="scrp", bufs=2) as scrp,
            tc.tile_pool(name="psa", bufs=2, space="PSUM") as psa,
            tc.tile_pool(name="psv", bufs=2, space="PSUM") as psv,
            tc.tile_pool(name="ebufp", bufs=2) as ebufp,
            tc.tile_pool(name="dumpp", bufs=2) as dumpp,
        ):
            sb_plhs = const.tile([K, NP], dt.bfloat16)
            sb_prhs = const.tile([K, NP], dt.bfloat16)
            sb_xrhs = const.tile([K, XS], dt.bfloat16)
            sb_pslhs = const.tile([K, PS_COLS], dt.bfloat16)
            sb_acc = const.tile([128, N_COLS], dt.float32)
            sb_tiny = const.tile([1, 1], dt.float32)
            sb_bias = const.tile([128, 1], dt.float32)

            # Warm the ACT exp table set (~2.7us) during the DMA prologue.
            nc.gpsimd.memset(sb_tiny[:], 0.0)
            nc.scalar.activation(
                sb_tiny[:], sb_tiny[:], mybir.ActivationFunctionType.Exp
            )

            # Input loads, in consumption order: the first waves need plhs
            # chunk 0 + the first xrhs half; the first t1 pair (at j=2)
            # needs pslhs + the first prhs super-group.
            pchunk = NP // N_PCHUNK
            nc.sync.dma_start(sb_bias[:], biasv[:])
            nc.sync.dma_start(sb_plhs[:, 0:pchunk], plhs[:, 0:pchunk])
            nc.sync.dma_start(sb_xrhs[:, 0:2048], xrhs[:, 0:2048])
            nc.sync.dma_start(sb_pslhs[:], pslhs[:])
            nc.sync.dma_start(sb_prhs[:, 0:2048], prhs[:, 0:2048])
            nc.sync.dma_start(sb_xrhs[:, 2048:XS], xrhs[:, 2048:XS])
            for i in range(1, N_PCHUNK):
                s = slice(i * pchunk, (i + 1) * pchunk)
                nc.sync.dma_start(sb_plhs[:, s], plhs[:, s])
            for i in range(1, 4):
                s = slice(i * 2048, (i + 1) * 2048)
                nc.sync.dma_start(sb_prhs[:, s], prhs[:, s])

            col = 0

            def z_matmuls(ps_t, src, slot, cstart):
                lhs = sb_plhs if src == "x" else sb_pslhs
                rhs = sb_xrhs if src == "x" else sb_prhs
                for q in range(W // 512):
                    nc.tensor.matmul(
                        ps_t[:, q * 512:(q + 1) * 512],
                        lhs[:, slot * 128:(slot + 1) * 128],
                        rhs[:, cstart + q * 512: cstart + (q + 1) * 512],
                    )

            def act_wave(src, slot, cstart):
                nonlocal col
                ps_t = psa.tile([128, W], dt.float32, tag="psa")
                z_matmuls(ps_t, src, slot, cstart)
                scr = scrp.tile([128, W], dt.float32, tag="scr")
                nc.scalar.activation(
                    scr[:],
                    ps_t[:],
                    mybir.ActivationFunctionType.Exp,
                    scale=inv_a,
                    bias=sb_bias[:],
                    accum_out=sb_acc[:, col:col + 1],
                )
                col += 1

            vstate = {"n": 0, "ebuf": None}

            def v_flush():
                nonlocal col
                n = vstate["n"]
                if not n:
                    return
                ebuf = vstate["ebuf"]
                dump = dumpp.tile([128, VBATCH * W], dt.bfloat16, tag="dump")
                nc.vector.tensor_scalar(
                    dump[:, : n * W], ebuf[:, : n * W].bitcast(dt.bfloat16),
                    1.0, None,
                    mybir.AluOpType.mult, mybir.AluOpType.add,
                    accum_out=sb_acc[:, col:col + 1],
                )
                col += 1
                vstate["n"] = 0
                vstate["ebuf"] = None

            def dve_wave(src, slot, cstart):
                ps_t = psv.tile([128, W], dt.float32, tag="psv")
                z_matmuls(ps_t, src, slot, cstart)
                if vstate["ebuf"] is None:
                    vstate["ebuf"] = ebufp.tile(
                        [128, VBATCH * W], dt.int16, tag="ebuf", name="ebuf"
                    )
                n = vstate["n"]
                nc.vector.tensor_scalar_max(
                    vstate["ebuf"][:, n * W:(n + 1) * W], ps_t[:], 0.0
                )
                vstate["n"] = n + 1
                if vstate["n"] == VBATCH:
                    v_flush()

            n_early = int(N_COLS * 0.7)
            if repeats == 0:  # timing-only baseline: I/O but no compute
                nc.gpsimd.memset(sb_acc[:], 0.0)
            for _ in range(repeats):  # repeats>1 is a timing-only variant
                col = 0
                shipped = False
                for eng, src, slot, cstart, _k, _w in _PLAN:
                    if eng == "A":
                        act_wave(src, slot, cstart)
                    else:
                        dve_wave(src, slot, cstart)
                    if not shipped and col >= n_early:
                        # ship the finished head of the accumulator while
                        # the tail is still computing
                        nc.sync.dma_start(acc_d[:, :col], sb_acc[:, :col])
                        shipped = True
                        n_early = col
                v_flush()
                if repeats:
                    assert col == N_COLS, (col, N_COLS)

            if repeats >= 1:
                nc.sync.dma_start(acc_d[:, n_early:], sb_acc[:, n_early:])
            else:
                nc.sync.dma_start(acc_d[:], sb_acc[:])

    nc.compile()
    return nc


def _split_hi_lo(v):
    vh = np.asarray(v, np.float32).astype(bf16)
    vl = (np.asarray(v, np.float32) - vh.astype(np.float32)).astype(bf16)
    return vh, vl


def _enc_lhsT(p):
    """p: [n, 16] f32 -> [K, n] bf16 stationary-side encoding (A-scaled)."""
    n = p.shape[0]
    q = (A_SCHR * np.ascontiguousarray(p, np.float64)).astype(np.float32)
    qh, ql = _split_hi_lo(q)
    p2 = (_b_schr() - A_SCHR * GAMMA * (p.astype(np.float64) ** 2).sum(-1)).astype(
        np.float32
    )
    p2h, p2l = _split_hi_lo(p2)
    out = np.empty((K, n), bf16)
    out[0:16] = qh.T
    out[16:32] = ql.T
    out[32:48] = qh.T
    out[48:64] = ql.T
    out[64] = p2h
    out[65] = p2l
    out[66] = bf16(1.0)
    out[67] = bf16(1.0)
    return out


def _enc_rhs(u):
    """u: [n, 16] f32 -> [K, n] bf16 moving-side encoding."""
    n = u.shape[0]
    uh, ul = _split_hi_lo(np.ascontiguousarray(u, np.float32))
    u2 = (-A_SCHR * GAMMA * (u.astype(np.float64) ** 2).sum(-1)).astype(np.float32)
    u2h, u2l = _split_hi_lo(u2)
    out = np.empty((K, n), bf16)
    out[0:16] = uh.T
    out[16:32] = uh.T
    out[32:48] = ul.T
    out[48:64] = ul.T
    out[64] = bf16(1.0)
    out[65] = bf16(1.0)
    out[66] = u2h
    out[67] = u2l
    return out


_lock = threading.Lock()
_cached_nc = None


def _get_nc():
    global _cached_nc
    with _lock:
        if _cached_nc is None:
            _cached_nc = _build_nc()
        return _cached_nc


def _make_in_maps(x, particles):
    plhs = _enc_lhsT(particles)
    prhs = _enc_rhs(particles)
    in_maps = []
    biasv = np.full((128, 1), -_b_schr() / A_SCHR, np.float32)
    for c in range(N_CORES):
        pairs = _t1_pairs(c)
        pslhs = np.concatenate(
            [plhs[:, r * 128:(r + 1) * 128] for r, _, _ in pairs], axis=1
        )
        in_maps.append(
            {
                "plhs": plhs,
                "prhs": prhs,
                "xrhs": _enc_rhs(x[c * XS:(c + 1) * XS]),
                "pslhs": np.ascontiguousarray(pslhs),
                "biasv": biasv,
            }
        )
    return in_maps


def _combine(results):
    t2_sum = 0.0
    t1_sum = 0.0
    for r in results:
        acc = r["acc"].astype(np.float64)
        colsums = acc.sum(axis=0)
        for i, (kind, w) in enumerate(_COLS):
            if kind == "t2":
                t2_sum += colsums[i]
            else:
                t1_sum += w * colsums[i]
    t1 = t1_sum / (float(NP) * NP)
    t2 = 2.0 * t2_sum / (float(NX) * NP)
    return np.float32(t1 - t2)


def kernel(x, particles):
    x = np.asarray(x, np.float32)
    particles = np.asarray(particles, np.float32)
    assert x.shape == (NX, D) and particles.shape == (NP, D)

    nc = _get_nc()
    in_maps = _make_in_maps(x, particles)
    res = bass_utils.run_bass_kernel_spmd(nc, in_maps, core_ids=list(range(N_CORES)))
    return _combine(res.results)


# revision 29
# speedup vs baseline: 1.3012x; 1.3012x over previous
"""Trainium2 Bass kernel for the ExpCloudMMD loss (v2: ACT+DVE split).

reference math (gamma = 0.5):
  t1 = mean_{j,k} exp(-g*||p_j - p_k||^2)            over [8192, 8192]
  t2 = 2/(Nx*Np) * sum_{i,j} exp(-g*||x_i - p_j||^2) over [32768, 8192]
  out = t1 - t2  (f32 scalar)

Strategy (8 cores, SPMD, no collectives):
  - t2: shard x rows 8-way (4096 rows/core vs all 8192 particles).
  - t1: coarse triangle over 2048-super-blocks, 160 (row-block, col-group)
    pairs dealt round-robin to cores via the pslhs input (program identical
    across cores); host doubles the off-diagonal sums.
  - The PE emits the *Schraudolph-scaled* exp argument directly:
        z = A*(p.x) - A*g*|p|^2 - A*g*|x|^2 + B,  A = 2^7/ln2,
        B = 127*2^7 - C  (C ~ 7.345 centers the bitcast sawtooth error)
    via one K=68 bf16 matmul per tile (4-way hi/lo split product + norm/
    bias channels).
  - Columns are split between two exp engines running concurrently:
      * ACT waves [128,1024]: activation(Exp, scale=1/A, bias=-B/A,
        accum_out) -- exact exp row-sums (the inverse affine is free).
      * DVE waves [128,1024]: tensor_scalar_max(int16_out, psum, 0)
        clamps and converts z to int16; bitcasting those bits as bf16 IS
        exp(a)*(1+eps), |eps|<~4%, mean-centered by C. A second
        tensor_scalar (bf16, 4x mode) accumulates them via accum_out.
    Wave pattern 9 ACT : 7 DVE balances the engines; PSUM = 2x1024 (ACT
    ping-pong) + 2x1024 (DVE ping-pong).
  - Per-wave row-sums land in one column of a [128, n_cols] SBUF
    accumulator; host does the final tiny weighted reduction in fp64.
    The sawtooth-centering constant C rides in the *input encoding*, so
    it can be recalibrated without recompiling.
"""

import threading

import ml_dtypes
import numpy as np

import concourse.bass as bass  # noqa: F401
import concourse.mybir as mybir
import concourse.tile as tile
from concourse import bacc, bass_utils

bf16 = ml_dtypes.bfloat16

GAMMA = 0.5
NX, NP, D = 32768, 16384 // 2, 16
N_CORES = 8
XS = NX // N_CORES     # 4096 x rows per core
K = 68                 # 4*16 (hi/lo product blocks) + 2 + 2 norm channels

A_SCHR = 128.0 / np.log(2.0)
# sawtooth centering: 7.3453 if the DVE fp32->int16 convert rounds to
# nearest, 6.8458 if it truncates. Rides in the *inputs* (encodings + the
# biasv tensor), so it can be recalibrated without recompiling.
C_SCHR = 7.3453


def _b_schr():
    return 127.0 * 128.0 - C_SCHR

# t1 coarse-triangle schedule: for col-super-group g (2048 particles), the
# computed row-blocks are the 16*(g+1) blocks of super-rows 0..g, dealt
# round-robin (r % 8) to cores -> per-core counts 2,4,6,8. Rows ascend, so
# per level g the first 2g pairs are off-diagonal (w=2) and the last 2 are
# diagonal (w=1) on EVERY core -- the program stays identical across cores.
T1_COUNTS = [2, 4, 6, 8]
N_T1_PAIRS = sum(T1_COUNTS)                    # 20 per core
PS_COLS = N_T1_PAIRS * 128                     # 2560 pslhs columns per core

N_PCHUNK = 8  # plhs load chunks for early compute start


def _t1_pairs(core):
    """[(row_block, col_group, weight)] for this core, in program order."""
    pairs = []
    for g in range(4):
        rows = [r for r in range(16 * (g + 1)) if r % N_CORES == core]
        assert len(rows) == T1_COUNTS[g]
        for r in rows:
            pairs.append((r, g, 1.0 if r // 16 == g else 2.0))
    return pairs


U = 512            # scheduling unit (columns); also DVE wave width
WA = 2 * U         # ACT wave width (two units)


def _t1_pair_units():
    """Per t1 pair (in _t1_pairs order): list of (col_start, weight) units.

    Off-diagonal pairs: full 2048 cols at w=2. The two diagonal pairs per
    level split the symmetric diagonal super-block into quadrants
    (UL + 2*UR + LR): the first diag row-block (i<8) does [0,1024) at w=1
    and [1024,2048) at w=2; the second (i>=8) only [1024,2048) at w=1.
    """
    out = []
    for g in range(4):
        base = g * 2048
        for _ in range(T1_COUNTS[g] - 2):
            out.append([(base + k * U, 2.0) for k in range(4)])
        out.append([(base, 1.0), (base + U, 1.0),
                    (base + 2 * U, 2.0), (base + 3 * U, 2.0)])
        out.append([(base + 2 * U, 1.0), (base + 3 * U, 1.0)])
    return out


def _unit_stream():
    """Yield (src, slot, col_start, kind, weight, dve_ok) 512-col units.

    src: 'x' (cross, rhs=xrhs, lhs=plhs[slot]) or 'p' (t1, rhs=prhs,
    lhs=pslhs[slot]). kind: 't2' or 't1'. Cross j-blocks are interleaved
    with t1 pairs (one pair after every 3rd j-block) so neither engine
    sees a long forced tail. Identical across cores by construction.

    t1 units are ACT-only: the diagonal super-blocks contain the exact
    self-pairs (a point mass at one sawtooth phase of the bitcast-exp
    error, which would not average out on the DVE path).
    """
    t1_seq = _t1_pair_units()
    t1_idx = 0
    for j in range(NP // 128):
        for c in range(4096 // U):
            yield ("x", j, c * U, "t2", 1.0, True)
        if (j % 3 == 2) and t1_idx < N_T1_PAIRS:
            for cs, w in t1_seq[t1_idx]:
                yield ("p", t1_idx, cs, "t1", w, False)
            t1_idx += 1
    while t1_idx < N_T1_PAIRS:
        for cs, w in t1_seq[t1_idx]:
            yield ("p", t1_idx, cs, "t1", w, False)
        t1_idx += 1


# modeled per-wave engine busy times used for the static split
T_ACT_WAVE = 1184.0    # [128,1024] exp+accum: (1024+172)/1.2 + 187
T_DVE_WAVE = 950.0     # [128,512] clamp+convert incl HW drain overhead


def _schedule():
    """Assign units to engines. Returns a list of emission ops:
      ("A", [(src, slot, cs, kind, w), ...])  -- one ACT wave (1-2 units)
      ("V", (src, slot, cs, kind, w))         -- one DVE wave (1 unit)
    ACT waves only combine units with identical (src, slot, kind, w) and
    contiguous columns, so each accumulator column stays pure."""
    units = list(_unit_stream())
    plan = []
    act_t = dve_t = 0.0
    i = 0
    while i < len(units):
        u0 = units[i]
        dve_ok = u0[5]
        if dve_ok and dve_t + T_DVE_WAVE <= act_t + T_ACT_WAVE:
            plan.append(("V", u0[:5]))
            dve_t += T_DVE_WAVE
            i += 1
            continue
        pair = []
        u1 = units[i + 1] if i + 1 < len(units) else None
        if (
            u1 is not None
            and u1[0] == u0[0] and u1[1] == u0[1]
            and u1[2] == u0[2] + U and u1[4] == u0[4]
        ):
            pair = [u0[:5], u1[:5]]
            i += 2
        else:
            pair = [u0[:5]]
            i += 1
        plan.append(("A", pair))
        act_t += T_ACT_WAVE if len(pair) == 2 else 757.0
    return plan


VRING = 4   # DVE waves buffered per ebuf ring tile (sum-matmul batch)
_PLAN = _schedule()
# accumulator columns: one per ACT wave, in emission order
_COLS = [(p[1][0][3], p[1][0][4]) for p in _PLAN if p[0] == "A"]
N_COLS = len(_COLS)
N_VSUM = 512  # width of the PE-side accumulation row for the DVE share


def _build_nc(repeats=1):
    nc = bacc.Bacc(
        "TRN2",
        target_bir_lowering=False,
        debug=False,
        enable_asserts=False,
        num_devices=N_CORES,
    )
    dt = mybir.dt
    plhs = nc.dram_tensor("plhs", [K, NP], dt.bfloat16, kind="ExternalInput").ap()
    prhs = nc.dram_tensor("prhs", [K, NP], dt.bfloat16, kind="ExternalInput").ap()
    xrhs = nc.dram_tensor("xrhs", [K, XS], dt.bfloat16, kind="ExternalInput").ap()
    pslhs = nc.dram_tensor("pslhs", [K, PS_COLS], dt.bfloat16, kind="ExternalInput").ap()
    biasv = nc.dram_tensor("biasv", [128, 1], dt.float32, kind="ExternalInput").ap()
    acc_d = nc.dram_tensor("acc", [128, N_COLS], dt.float32, kind="ExternalOutput").ap()
    accv_d = nc.dram_tensor("accv", [1, N_VSUM], dt.float32, kind="ExternalOutput").ap()

    inv_a = float(1.0 / A_SCHR)

    with tile.TileContext(nc) as tc:
        with (
            tc.tile_pool(name="const", bufs=1) as const,
            tc.tile_pool(name="psa", bufs=2, space="PSUM") as psa,
            tc.tile_pool(name="psv", bufs=2, space="PSUM") as psv,
            tc.tile_pool(name="psacc", bufs=1, space="PSUM") as psacc,
            tc.tile_pool(name="ebufp", bufs=2) as ebufp,
        ):
            sb_plhs = const.tile([K, NP], dt.bfloat16)
            sb_prhs = const.tile([K, NP], dt.bfloat16)
            sb_xrhs = const.tile([K, XS], dt.bfloat16)
            sb_pslhs = const.tile([K, PS_COLS], dt.bfloat16)
            sb_acc = const.tile([128, N_COLS], dt.float32)
            sb_vrow = const.tile([1, N_VSUM], dt.float32)
            sb_ones = const.tile([128, 1], dt.bfloat16)
            sb_closer = const.tile([128, N_VSUM], dt.bfloat16)
            sb_tiny = const.tile([1, 1], dt.float32)
            sb_bias = const.tile([128, 1], dt.float32)
            nc.vector.memset(sb_ones[:], 1.0)
            nc.vector.memset(sb_closer[:].bitcast(dt.uint16), 0)

            # Warm the ACT exp table set (~2.7us) during the DMA prologue.
            nc.gpsimd.memset(sb_tiny[:], 0.0)
            nc.scalar.activation(
                sb_tiny[:], sb_tiny[:], mybir.ActivationFunctionType.Exp
            )

            # Input loads, in consumption order: the first waves need plhs
            # chunk 0 + the first xrhs half; the first t1 pair (at j=2)
            # needs pslhs + the first prhs super-group.
            pchunk = NP // N_PCHUNK
            nc.sync.dma_start(sb_bias[:], biasv[:])
            nc.sync.dma_start(sb_plhs[:, 0:pchunk], plhs[:, 0:pchunk])
            nc.sync.dma_start(sb_xrhs[:, 0:2048], xrhs[:, 0:2048])
            nc.sync.dma_start(sb_pslhs[:], pslhs[:])
            nc.sync.dma_start(sb_prhs[:, 0:2048], prhs[:, 0:2048])
            nc.sync.dma_start(sb_xrhs[:, 2048:XS], xrhs[:, 2048:XS])
            for i in range(1, N_PCHUNK):
                s = slice(i * pchunk, (i + 1) * pchunk)
                nc.sync.dma_start(sb_plhs[:, s], plhs[:, s])
            for i in range(1, 4):
                s = slice(i * 2048, (i + 1) * 2048)
                nc.sync.dma_start(sb_prhs[:, s], prhs[:, s])

            col = 0
            ps_vsum = psacc.tile([128, N_VSUM], dt.float32)

            def z_matmuls(ps_t, units):
                for q, (src, slot, cstart, _k, _w) in enumerate(units):
                    lhs = sb_plhs if src == "x" else sb_pslhs
                    rhs = sb_xrhs if src == "x" else sb_prhs
                    nc.tensor.matmul(
                        ps_t[:, q * U:(q + 1) * U],
                        lhs[:, slot * 128:(slot + 1) * 128],
                        rhs[:, cstart: cstart + U],
                    )

            def act_wave(units):
                nonlocal col
                w = len(units) * U
                ps_t = psa.tile([128, WA], dt.float32, tag="psa")
                z_matmuls(ps_t, units)
                # in-place PSUM destination: ScE is closer to PSUM (172- vs
                # 224-cycle access bubble) and no scratch SBUF tile needed
                nc.scalar.activation(
                    ps_t[:, :w],
                    ps_t[:, :w],
                    mybir.ActivationFunctionType.Exp,
                    scale=inv_a,
                    bias=sb_bias[:],
                    accum_out=sb_acc[:, col:col + 1],
                )
                col += 1

            vstate = {"n": 0, "ebuf": None, "started": False}

            def v_flush():
                """PE ones-matmuls reduce the buffered bitcast-exp values,
                accumulating into the persistent [1, N_VSUM] PSUM row."""
                n = vstate["n"]
                if not n:
                    return
                ebuf = vstate["ebuf"]
                for q in range(n):
                    nc.tensor.matmul(
                        ps_vsum[0:1, :N_VSUM],
                        sb_ones[:],
                        ebuf[:, q * U:(q + 1) * U].bitcast(dt.bfloat16),
                        start=not vstate["started"],
                        stop=False,
                        skip_group_check=True,
                    )
                    vstate["started"] = True
                vstate["n"] = 0
                vstate["ebuf"] = None

            def dve_wave(unit):
                ps_t = psv.tile([128, U], dt.float32, tag="psv")
                z_matmuls(ps_t, [unit])
                if vstate["ebuf"] is None:
                    vstate["ebuf"] = ebufp.tile(
                        [128, VRING * U], dt.int16, tag="ebuf", name="ebuf"
                    )
                n = vstate["n"]
                nc.vector.tensor_scalar_max(
                    vstate["ebuf"][:, n * U:(n + 1) * U], ps_t[:], 0.0
                )
                vstate["n"] = n + 1
                if vstate["n"] == VRING:
                    v_flush()

            n_early = int(N_COLS * 0.7)
            if repeats == 0:  # timing-only baseline: I/O but no compute
                nc.gpsimd.memset(sb_acc[:], 0.0)
                nc.gpsimd.memset(sb_vrow[:], 0.0)
            for _ in range(repeats):  # repeats>1 is a timing-only variant
                col = 0
                shipped = False
                vstate["started"] = False
                for eng, payload in _PLAN:
                    if eng == "A":
                        act_wave(payload)
                    else:
                        dve_wave(payload)
                    if not shipped and col >= n_early:
                        # ship the finished head of the accumulator while
                        # the tail is still computing
                        nc.sync.dma_start(acc_d[:, :col], sb_acc[:, :col])
                        shipped = True
                        n_early = col
                v_flush()
                # close the accumulation group (adds zeros) and read the
                # PE-side sums
                nc.tensor.matmul(
                    ps_vsum[0:1, :N_VSUM], sb_ones[:], sb_closer[:],
                    start=False, stop=True, skip_group_check=True,
                )
                nc.scalar.copy(sb_vrow[:], ps_vsum[0:1, :N_VSUM])
                if repeats:
                    assert col == N_COLS, (col, N_COLS)

            if repeats >= 1:
                nc.sync.dma_start(acc_d[:, n_early:], sb_acc[:, n_early:])
            else:
                nc.sync.dma_start(acc_d[:], sb_acc[:])
            nc.sync.dma_start(accv_d[:], sb_vrow[:])

    nc.compile()
    return nc


def _split_hi_lo(v):
    vh = np.asarray(v, np.float32).astype(bf16)
    vl = (np.asarray(v, np.float32) - vh.astype(np.float32)).astype(bf16)
    return vh, vl


def _enc_lhsT(p):
    """p: [n, 16] f32 -> [K, n] bf16 stationary-side encoding (A-scaled)."""
    n = p.shape[0]
    q = (A_SCHR * np.ascontiguousarray(p, np.float64)).astype(np.float32)
    qh, ql = _split_hi_lo(q)
    p2 = (_b_schr() - A_SCHR * GAMMA * (p.astype(np.float64) ** 2).sum(-1)).astype(
        np.float32
    )
    p2h, p2l = _split_hi_lo(p2)
    out = np.empty((K, n), bf16)
    out[0:16] = qh.T
    out[16:32] = ql.T
    out[32:48] = qh.T
    out[48:64] = ql.T
    out[64] = p2h
    out[65] = p2l
    out[66] = bf16(1.0)
    out[67] = bf16(1.0)
    return out


def _enc_rhs(u):
    """u: [n, 16] f32 -> [K, n] bf16 moving-side encoding."""
    n = u.shape[0]
    uh, ul = _split_hi_lo(np.ascontiguousarray(u, np.float32))
    u2 = (-A_SCHR * GAMMA * (u.astype(np.float64) ** 2).sum(-1)).astype(np.float32)
    u2h, u2l = _split_hi_lo(u2)
    out = np.empty((K, n), bf16)
    out[0:16] = uh.T
    out[16:32] = uh.T
    out[32:48] = ul.T
    out[48:64] = ul.T
    out[64] = bf16(1.0)
    out[65] = bf16(1.0)
    out[66] = u2h
    out[67] = u2l
    return out


_lock = threading.Lock()
_cached_nc = None


def _get_nc():
    global _cached_nc
    with _lock:
        if _cached_nc is None:
            _cached_nc = _build_nc()
        return _cached_nc


def _make_in_maps(x, particles):
    plhs = _enc_lhsT(particles)
    prhs = _enc_rhs(particles)
    in_maps = []
    biasv = np.full((128, 1), -_b_schr() / A_SCHR, np.float32)
    for c in range(N_CORES):
        pairs = _t1_pairs(c)
        pslhs = np.concatenate(
            [plhs[:, r * 128:(r + 1) * 128] for r, _, _ in pairs], axis=1
        )
        in_maps.append(
            {
                "plhs": plhs,
                "prhs": prhs,
                "xrhs": _enc_rhs(x[c * XS:(c + 1) * XS]),
                "pslhs": np.ascontiguousarray(pslhs),
                "biasv": biasv,
            }
        )
    return in_maps


def _combine(results):
    t2_sum = 0.0
    t1_sum = 0.0
    for r in results:
        acc = r["acc"].astype(np.float64)
        colsums = acc.sum(axis=0)
        for i, (kind, w) in enumerate(_COLS):
            if kind == "t2":
                t2_sum += colsums[i]
            else:
                t1_sum += w * colsums[i]
        t2_sum += r["accv"].astype(np.float64).sum()
    t1 = t1_sum / (float(NP) * NP)
    t2 = 2.0 * t2_sum / (float(NX) * NP)
    return np.float32(t1 - t2)


def kernel(x, particles):
    x = np.asarray(x, np.float32)
    particles = np.asarray(particles, np.float32)
    assert x.shape == (NX, D) and particles.shape == (NP, D)

    nc = _get_nc()
    in_maps = _make_in_maps(x, particles)
    res = bass_utils.run_bass_kernel_spmd(nc, in_maps, core_ids=list(range(N_CORES)))
    return _combine(res.results)


# revision 30
# speedup vs baseline: 17.3093x; 13.3025x over previous
"""Trainium2 Bass kernel for the ExpCloudMMD loss (v2: ACT+DVE split).

reference math (gamma = 0.5):
  t1 = mean_{j,k} exp(-g*||p_j - p_k||^2)            over [8192, 8192]
  t2 = 2/(Nx*Np) * sum_{i,j} exp(-g*||x_i - p_j||^2) over [32768, 8192]
  out = t1 - t2  (f32 scalar)

Strategy (8 cores, SPMD, no collectives):
  - t2: shard x rows 8-way (4096 rows/core vs all 8192 particles).
  - t1: coarse triangle over 2048-super-blocks, 160 (row-block, col-group)
    pairs dealt round-robin to cores via the pslhs input (program identical
    across cores); host doubles the off-diagonal sums.
  - The PE emits the *Schraudolph-scaled* exp argument directly:
        z = A*(p.x) - A*g*|p|^2 - A*g*|x|^2 + B,  A = 2^7/ln2,
        B = 127*2^7 - C  (C ~ 7.345 centers the bitcast sawtooth error)
    via one K=68 bf16 matmul per tile (4-way hi/lo split product + norm/
    bias channels).
  - Columns are split between two exp engines running concurrently:
      * ACT waves [128,1024]: activation(Exp, scale=1/A, bias=-B/A,
        accum_out) -- exact exp row-sums (the inverse affine is free).
      * DVE waves [128,1024]: tensor_scalar_max(int16_out, psum, 0)
        clamps and converts z to int16; bitcasting those bits as bf16 IS
        exp(a)*(1+eps), |eps|<~4%, mean-centered by C. A second
        tensor_scalar (bf16, 4x mode) accumulates them via accum_out.
    Wave pattern 9 ACT : 7 DVE balances the engines; PSUM = 2x1024 (ACT
    ping-pong) + 2x1024 (DVE ping-pong).
  - Per-wave row-sums land in one column of a [128, n_cols] SBUF
    accumulator; host does the final tiny weighted reduction in fp64.
    The sawtooth-centering constant C rides in the *input encoding*, so
    it can be recalibrated without recompiling.
"""

import threading

import ml_dtypes
import numpy as np

import concourse.bass as bass  # noqa: F401
import concourse.mybir as mybir
import concourse.tile as tile
from concourse import bacc, bass_utils

bf16 = ml_dtypes.bfloat16

GAMMA = 0.5
NX, NP, D = 32768, 16384 // 2, 16
N_CORES = 8
XS = NX // N_CORES     # 4096 x rows per core
K = 68                 # 4*16 (hi/lo product blocks) + 2 + 2 norm channels

A_SCHR = 128.0 / np.log(2.0)
# sawtooth centering: 7.3453 if the DVE fp32->int16 convert rounds to
# nearest, 6.8458 if it truncates. Rides in the *inputs* (encodings + the
# biasv tensor), so it can be recalibrated without recompiling.
C_SCHR = 7.3453


def _b_schr():
    return 127.0 * 128.0 - C_SCHR

# t1 coarse-triangle schedule: for col-super-group g (2048 particles), the
# computed row-blocks are the 16*(g+1) blocks of super-rows 0..g, dealt
# round-robin (r % 8) to cores -> per-core counts 2,4,6,8. Rows ascend, so
# per level g the first 2g pairs are off-diagonal (w=2) and the last 2 are
# diagonal (w=1) on EVERY core -- the program stays identical across cores.
T1_COUNTS = [2, 4, 6, 8]
N_T1_PAIRS = sum(T1_COUNTS)                    # 20 per core
PS_COLS = N_T1_PAIRS * 128                     # 2560 pslhs columns per core

N_PCHUNK = 8  # plhs load chunks for early compute start


def _t1_pairs(core):
    """[(row_block, col_group, weight)] for this core, in program order."""
    pairs = []
    for g in range(4):
        rows = [r for r in range(16 * (g + 1)) if r % N_CORES == core]
        assert len(rows) == T1_COUNTS[g]
        for r in rows:
            pairs.append((r, g, 1.0 if r // 16 == g else 2.0))
    return pairs


U = 512            # scheduling unit (columns); also DVE wave width
WA = 2 * U         # ACT wave width (two units)


def _t1_pair_units():
    """Per t1 pair (in _t1_pairs order): list of (col_start, weight) units.

    Off-diagonal pairs: full 2048 cols at w=2. The two diagonal pairs per
    level split the symmetric diagonal super-block into quadrants
    (UL + 2*UR + LR): the first diag row-block (i<8) does [0,1024) at w=1
    and [1024,2048) at w=2; the second (i>=8) only [1024,2048) at w=1.
    """
    out = []
    for g in range(4):
        base = g * 2048
        for _ in range(T1_COUNTS[g] - 2):
            out.append([(base + k * U, 2.0) for k in range(4)])
        out.append([(base, 1.0), (base + U, 1.0),
                    (base + 2 * U, 2.0), (base + 3 * U, 2.0)])
        out.append([(base + 2 * U, 1.0), (base + 3 * U, 1.0)])
    return out


def _unit_stream():
    """Yield (src, slot, col_start, kind, weight, dve_ok) 512-col units.

    src: 'x' (cross, rhs=xrhs, lhs=plhs[slot]) or 'p' (t1, rhs=prhs,
    lhs=pslhs[slot]). kind: 't2' or 't1'. Cross j-blocks are interleaved
    with t1 pairs (one pair after every 3rd j-block) so neither engine
    sees a long forced tail. Identical across cores by construction.

    t1 units are ACT-only: the diagonal super-blocks contain the exact
    self-pairs (a point mass at one sawtooth phase of the bitcast-exp
    error, which would not average out on the DVE path).
    """
    t1_seq = _t1_pair_units()
    t1_idx = 0
    for j in range(NP // 128):
        for c in range(4096 // U):
            yield ("x", j, c * U, "t2", 1.0, True)
        if (j % 3 == 2) and t1_idx < N_T1_PAIRS:
            for cs, w in t1_seq[t1_idx]:
                yield ("p", t1_idx, cs, "t1", w, False)
            t1_idx += 1
    while t1_idx < N_T1_PAIRS:
        for cs, w in t1_seq[t1_idx]:
            yield ("p", t1_idx, cs, "t1", w, False)
        t1_idx += 1


# modeled per-wave engine busy times used for the static split.
# T_DVE_WAVE is HW-fitted: DVE ops pay a pipe-drain tax of roughly their
# own duration (measured 437us/336us on DVE-heavy variants vs 219/229
# modeled), so a [128,512] clamp+convert costs ~2.2x its stream time.
T_ACT_WAVE = 1184.0    # [128,1024] exp+accum: (1024+172)/1.2 + 187
T_DVE_WAVE = 1250.0    # [128,512] clamp+convert incl drain tax


def _schedule():
    """Assign units to engines. Returns a list of emission ops:
      ("A", [(src, slot, cs, kind, w), ...])  -- one ACT wave (1-2 units)
      ("V", (src, slot, cs, kind, w))         -- one DVE wave (1 unit)
    ACT waves only combine units with identical (src, slot, kind, w) and
    contiguous columns, so each accumulator column stays pure."""
    units = list(_unit_stream())
    plan = []
    act_t = dve_t = 0.0
    i = 0
    while i < len(units):
        u0 = units[i]
        dve_ok = u0[5]
        if dve_ok and dve_t + T_DVE_WAVE <= act_t + T_ACT_WAVE:
            plan.append(("V", u0[:5]))
            dve_t += T_DVE_WAVE
            i += 1
            continue
        pair = []
        u1 = units[i + 1] if i + 1 < len(units) else None
        if (
            u1 is not None
            and u1[0] == u0[0] and u1[1] == u0[1]
            and u1[2] == u0[2] + U and u1[4] == u0[4]
        ):
            pair = [u0[:5], u1[:5]]
            i += 2
        else:
            pair = [u0[:5]]
            i += 1
        plan.append(("A", pair))
        act_t += T_ACT_WAVE if len(pair) == 2 else 757.0
    return plan


VRING = 4   # DVE waves buffered per ebuf ring tile (sum-matmul batch)
_PLAN = _schedule()
# accumulator columns: one per ACT wave, in emission order
_COLS = [(p[1][0][3], p[1][0][4]) for p in _PLAN if p[0] == "A"]
N_COLS = len(_COLS)
N_VSUM = 512  # width of the PE-side accumulation row for the DVE share


def _build_nc(repeats=1):
    nc = bacc.Bacc(
        "TRN2",
        target_bir_lowering=False,
        debug=False,
        enable_asserts=False,
        num_devices=N_CORES,
    )
    dt = mybir.dt
    plhs = nc.dram_tensor("plhs", [K, NP], dt.bfloat16, kind="ExternalInput").ap()
    prhs = nc.dram_tensor("prhs", [K, NP], dt.bfloat16, kind="ExternalInput").ap()
    xrhs = nc.dram_tensor("xrhs", [K, XS], dt.bfloat16, kind="ExternalInput").ap()
    pslhs = nc.dram_tensor("pslhs", [K, PS_COLS], dt.bfloat16, kind="ExternalInput").ap()
    biasv = nc.dram_tensor("biasv", [128, 1], dt.float32, kind="ExternalInput").ap()
    acc_d = nc.dram_tensor("acc", [128, N_COLS], dt.float32, kind="ExternalOutput").ap()
    accv_d = nc.dram_tensor("accv", [1, N_VSUM], dt.float32, kind="ExternalOutput").ap()

    inv_a = float(1.0 / A_SCHR)

    with tile.TileContext(nc) as tc:
        with (
            tc.tile_pool(name="const", bufs=1) as const,
            tc.tile_pool(name="psa", bufs=2, space="PSUM") as psa,
            tc.tile_pool(name="psv", bufs=2, space="PSUM") as psv,
            tc.tile_pool(name="psacc", bufs=1, space="PSUM") as psacc,
            tc.tile_pool(name="ebufp", bufs=2) as ebufp,
        ):
            sb_plhs = const.tile([K, NP], dt.bfloat16)
            sb_prhs = const.tile([K, NP], dt.bfloat16)
            sb_xrhs = const.tile([K, XS], dt.bfloat16)
            sb_pslhs = const.tile([K, PS_COLS], dt.bfloat16)
            sb_acc = const.tile([128, N_COLS], dt.float32)
            sb_vrow = const.tile([1, N_VSUM], dt.float32)
            sb_ones = const.tile([128, 1], dt.bfloat16)
            sb_closer = const.tile([128, N_VSUM], dt.bfloat16)
            sb_tiny = const.tile([1, 1], dt.float32)
            sb_bias = const.tile([128, 1], dt.float32)
            nc.vector.memset(sb_ones[:], 1.0)
            nc.vector.memset(sb_closer[:].bitcast(dt.uint16), 0)

            # Warm the ACT exp table set (~2.7us) during the DMA prologue.
            nc.gpsimd.memset(sb_tiny[:], 0.0)
            nc.scalar.activation(
                sb_tiny[:], sb_tiny[:], mybir.ActivationFunctionType.Exp
            )

            # Input loads, in consumption order: the first waves need plhs
            # chunk 0 + the first xrhs half; the first t1 pair (at j=2)
            # needs pslhs + the first prhs super-group.
            pchunk = NP // N_PCHUNK
            nc.sync.dma_start(sb_bias[:], biasv[:])
            nc.sync.dma_start(sb_plhs[:, 0:pchunk], plhs[:, 0:pchunk])
            nc.sync.dma_start(sb_xrhs[:, 0:2048], xrhs[:, 0:2048])
            nc.sync.dma_start(sb_pslhs[:], pslhs[:])
            nc.sync.dma_start(sb_prhs[:, 0:2048], prhs[:, 0:2048])
            nc.sync.dma_start(sb_xrhs[:, 2048:XS], xrhs[:, 2048:XS])
            for i in range(1, N_PCHUNK):
                s = slice(i * pchunk, (i + 1) * pchunk)
                nc.sync.dma_start(sb_plhs[:, s], plhs[:, s])
            for i in range(1, 4):
                s = slice(i * 2048, (i + 1) * 2048)
                nc.sync.dma_start(sb_prhs[:, s], prhs[:, s])

            col = 0
            ps_vsum = psacc.tile([128, N_VSUM], dt.float32)

            def z_matmuls(ps_t, units):
                for q, (src, slot, cstart, _k, _w) in enumerate(units):
                    lhs = sb_plhs if src == "x" else sb_pslhs
                    rhs = sb_xrhs if src == "x" else sb_prhs
                    nc.tensor.matmul(
                        ps_t[:, q * U:(q + 1) * U],
                        lhs[:, slot * 128:(slot + 1) * 128],
                        rhs[:, cstart: cstart + U],
                    )

            def act_wave(units):
                nonlocal col
                w = len(units) * U
                ps_t = psa.tile([128, WA], dt.float32, tag="psa")
                z_matmuls(ps_t, units)
                # in-place PSUM destination: ScE is closer to PSUM (172- vs
                # 224-cycle access bubble) and no scratch SBUF tile needed
                nc.scalar.activation(
                    ps_t[:, :w],
                    ps_t[:, :w],
                    mybir.ActivationFunctionType.Exp,
                    scale=inv_a,
                    bias=sb_bias[:],
                    accum_out=sb_acc[:, col:col + 1],
                )
                col += 1

            vstate = {"n": 0, "ebuf": None, "started": False}

            def v_flush():
                """PE ones-matmuls reduce the buffered bitcast-exp values,
                accumulating into the persistent [1, N_VSUM] PSUM row."""
                n = vstate["n"]
                if not n:
                    return
                ebuf = vstate["ebuf"]
                for q in range(n):
                    nc.tensor.matmul(
                        ps_vsum[0:1, :N_VSUM],
                        sb_ones[:],
                        ebuf[:, q * U:(q + 1) * U].bitcast(dt.bfloat16),
                        start=not vstate["started"],
                        stop=False,
                        skip_group_check=True,
                    )
                    vstate["started"] = True
                vstate["n"] = 0
                vstate["ebuf"] = None

            def dve_wave(unit):
                ps_t = psv.tile([128, U], dt.float32, tag="psv")
                z_matmuls(ps_t, [unit])
                if vstate["ebuf"] is None:
                    vstate["ebuf"] = ebufp.tile(
                        [128, VRING * U], dt.int16, tag="ebuf", name="ebuf"
                    )
                n = vstate["n"]
                nc.vector.tensor_scalar_max(
                    vstate["ebuf"][:, n * U:(n + 1) * U], ps_t[:], 0.0
                )
                vstate["n"] = n + 1
                if vstate["n"] == VRING:
                    v_flush()

            n_early = int(N_COLS * 0.7)
            if repeats == 0:  # timing-only baseline: I/O but no compute
                nc.gpsimd.memset(sb_acc[:], 0.0)
                nc.gpsimd.memset(sb_vrow[:], 0.0)
            for _ in range(repeats):  # repeats>1 is a timing-only variant
                col = 0
                shipped = False
                vstate["started"] = False
                for eng, payload in _PLAN:
                    if eng == "A":
                        act_wave(payload)
                    else:
                        dve_wave(payload)
                    if not shipped and col >= n_early:
                        # ship the finished head of the accumulator while
                        # the tail is still computing
                        nc.sync.dma_start(acc_d[:, :col], sb_acc[:, :col])
                        shipped = True
                        n_early = col
                v_flush()
                # close the accumulation group (adds zeros) and read the
                # PE-side sums
                nc.tensor.matmul(
                    ps_vsum[0:1, :N_VSUM], sb_ones[:], sb_closer[:],
                    start=False, stop=True, skip_group_check=True,
                )
                nc.scalar.copy(sb_vrow[:], ps_vsum[0:1, :N_VSUM])
                if repeats:
                    assert col == N_COLS, (col, N_COLS)

            if repeats >= 1:
                nc.sync.dma_start(acc_d[:, n_early:], sb_acc[:, n_early:])
            else:
                nc.sync.dma_start(acc_d[:], sb_acc[:])
            nc.sync.dma_start(accv_d[:], sb_vrow[:])

    nc.compile()
    return nc


def _split_hi_lo(v):
    vh = np.asarray(v, np.float32).astype(bf16)
    vl = (np.asarray(v, np.float32) - vh.astype(np.float32)).astype(bf16)
    return vh, vl


def _enc_lhsT(p):
    """p: [n, 16] f32 -> [K, n] bf16 stationary-side encoding (A-scaled)."""
    n = p.shape[0]
    q = (A_SCHR * np.ascontiguousarray(p, np.float64)).astype(np.float32)
    qh, ql = _split_hi_lo(q)
    p2 = (_b_schr() - A_SCHR * GAMMA * (p.astype(np.float64) ** 2).sum(-1)).astype(
        np.float32
    )
    p2h, p2l = _split_hi_lo(p2)
    out = np.empty((K, n), bf16)
    out[0:16] = qh.T
    out[16:32] = ql.T
    out[32:48] = qh.T
    out[48:64] = ql.T
    out[64] = p2h
    out[65] = p2l
    out[66] = bf16(1.0)
    out[67] = bf16(1.0)
    return out


def _enc_rhs(u):
    """u: [n, 16] f32 -> [K, n] bf16 moving-side encoding."""
    n = u.shape[0]
    uh, ul = _split_hi_lo(np.ascontiguousarray(u, np.float32))
    u2 = (-A_SCHR * GAMMA * (u.astype(np.float64) ** 2).sum(-1)).astype(np.float32)
    u2h, u2l = _split_hi_lo(u2)
    out = np.empty((K, n), bf16)
    out[0:16] = uh.T
    out[16:32] = uh.T
    out[32:48] = ul.T
    out[48:64] = ul.T
    out[64] = bf16(1.0)
    out[65] = bf16(1.0)
    out[66] = u2h
    out[67] = u2l
    return out


_lock = threading.Lock()
_cached_nc = None


def _get_nc():
    global _cached_nc
    with _lock:
        if _cached_nc is None:
            _cached_nc = _build_nc()
        return _cached_nc


def _make_in_maps(x, particles):
    plhs = _enc_lhsT(particles)
    prhs = _enc_rhs(particles)
    in_maps = []
    biasv = np.full((128, 1), -_b_schr() / A_SCHR, np.float32)
    for c in range(N_CORES):
        pairs = _t1_pairs(c)
        pslhs = np.concatenate(
            [plhs[:, r * 128:(r + 1) * 128] for r, _, _ in pairs], axis=1
        )
        in_maps.append(
            {
                "plhs": plhs,
                "prhs": prhs,
                "xrhs": _enc_rhs(x[c * XS:(c + 1) * XS]),
                "pslhs": np.ascontiguousarray(pslhs),
                "biasv": biasv,
            }
        )
    return in_maps


def _combine(results):
    t2_sum = 0.0
    t1_sum = 0.0
    for r in results:
        acc = r["acc"].astype(np.float64)
        colsums = acc.sum(axis=0)
        for i, (kind, w) in enumerate(_COLS):
            if kind == "t2":
                t2_sum += colsums[i]
            else:
                t1_sum += w * colsums[i]
        t2_sum += r["accv"].astype(np.float64).sum()
    t1 = t1_sum / (float(NP) * NP)
    t2 = 2.0 * t2_sum / (float(NX) * NP)
    return np.float32(t1 - t2)


def kernel(x, particles):
    x = np.asarray(x, np.float32)
    particles = np.asarray(particles, np.float32)
    assert x.shape == (NX, D) and particles.shape == (NP, D)

    nc = _get_nc()
    in_maps = _make_in_maps(x, particles)
    res = bass_utils.run_bass_kernel_spmd(nc, in_maps, core_ids=list(range(N_CORES)))
    return _combine(res.results)
